# revision 44
# baseline (speedup 1.0000x reference)
"""BiLSTM Trainium2 kernel: B=64, T=512, D=256, H=256, 8 NeuronCores.

Sharding: batch 8-way (8 sequences per core). Each core runs BOTH
directions (forward + backward) as two independent recurrent chains,
staggered half a step so one chain's elementwise hides under the other
chain's matmuls. Total time = 512 x chain-loop-latency (~2.17us), so
every change targets the serial loop:
  12 MM issue (324) -> psum drain (167) -> hop -> sig(f,g,i) (~300)
  -> fused fc|tmp' custom-DVE op (~230) -> Cnew (~200) -> tanh(2C)
  (~340) -> h-mul (~200) -> next burst.

Design notes:
  - JIT projection into PSUM: Wih.T@x + bias is matmul'd DIRECTLY into
    the recurrence PSUM banks (8 steps/bank, j-major: col = j*64 +
    toff*8 + b; all 8 banks used, 4 per dir), paced ~3 banks ahead, ~3
    matmuls per step, each pinned between the surrounding bursts with
    ordering-only deps (otherwise the Tile scheduler bunches a whole
    bank into a 1us PE wall). The recurrence matmuls accumulate on top
    and sigmoid reads the bank strided. PSUM start=True clears the
    accumulate-flags of the WHOLE bank -> exactly one start per bank
    fill. The bias rides a normal K=128 matmul (bias in lhsT row 0,
    rhs ones in row 0) because a K=1 matmul between K=128 ones breaks
    LDWEIGHTS/MM overlap (PE tile reconfig, ~230ns per matmul).
  - Gate chunk order (f,g,i,o): sigmoid(f,g,i) starts after 12 of the
    16 Whh matmuls ([kk0 j0-5][kk1 j0-5][kk0 j6-7][kk1 j6-7]);
    sigmoid(o), only needed by the h-mul ~900ns later, runs after the
    remaining 4.
  - Cell state tracked as C = c/2 in the act tile's C slot:
    C = sig(f)*C + (sig(2g) - 0.5)*sig(i), with both products computed
    by ONE custom DVE op (LSTM_FG_PROD_ANT, per-NEFF table):
    out = select(k<16, a, a-0.5)*b over in0=[sig f|sig 2g] and the
    2-block strided view in1=[C|sig i]. tanh(c) = tanh(2C) rides the
    ACT tanh's free input scale. act layout [f g i C o], double
    buffered per dir so step t's Cnew lands in step t+1's buffer.
  - h lives directly in the bf16 output staging tiles (DMA'd out every
    32 steps); the burst reads it as [kk0|kk1] halves, h-mul split so
    the kk0 half of the next burst starts one DVE-op earlier.
  - Upfront DMA descriptor order matters (~650ns serial each on Sync,
    and the grouping measurably shifts the steady period): keep
    xt-chunk0(f), wih(f), bias(f), xt-chunk0(b), wih(b), bias(b),
    whh(f), whh(b), then chunk1; later xt chunks paced into the loop.
  - Measured dead ends: fp8e4 DoubleRow (256-row strided LDWEIGHTS
    doesn't overlap the MM stream: 123ns/MM vs bf16's 27), fc on
    GPSIMD (q7+sem latency), merged 64-col sigmoid (+108ns path beats
    the ACT relief), sigmoid/tanh via DVE (no DIVIDE on trn2 DVE).
"""

import sys

for _p in ("/opt/trn_rl_repo",):
    if _p not in sys.path:
        sys.path.insert(0, _p)

import numpy as np
import ml_dtypes

import concourse.bass as bass
import concourse.mybir as mybir
import concourse.tile as tile
from concourse.tile import add_dep_helper
from concourse import bacc
from concourse.bass_utils import run_bass_kernel_spmd

import concourse.dve_ops as dve_ops
from concourse.dve_ops import DveOp
from concourse.dve_spec import Spec, Src0, Src1, C0, C1, Idx, select, lower, _has_src1
from concourse.dve_uop import DveOpSpec


def _register_fg_prod():
    """Custom DVE op (per-NEFF table, no firmware change):
        out[p,k] = (k < s1 ? in0[p,k] : in0[p,k] - s0) * in1[p,k]
    Fuses the LSTM round-1 products fc = sig(f)*C and
    tmp' = (sig(2g) - 0.5)*sig(i) into one DVE instruction."""
    name = "LSTM_FG_PROD_ANT"
    for op in dve_ops.OPS:
        if op.name == name:
            return op
    body = select(Idx < C1, Src0, Src0 - C0) * Src1

    def ref(in0, in1, s0, s1, imm2):
        k = np.arange(in0.shape[-1], dtype=np.float32)
        return np.where(k < s1, in0, in0 - s0) * in1

    spec = Spec(body=body, reference=ref)
    row = max(dve_ops._SUB_OPCODE_FOR_NAME.values()) + 1
    assert row < 0x20
    shas = {}
    for ver in ("v3", "v4"):
        u = lower(spec, ver=ver)
        shas[ver] = DveOpSpec(
            name=name, opcode=row, uops=u, rd1_en=_has_src1(spec)
        ).sha(ver)
    op = DveOp(name, spec, subdim=False, uops_sha=shas,
               perf_en={"v3": True, "v4": True})
    dve_ops.OPS.append(op)
    dve_ops.CUSTOM_DVE_SPECS[name] = spec
    dve_ops._SUB_OPCODE_FOR_NAME[name] = row
    return op


FG_PROD = _register_fg_prod()

B, T, D, H = 64, 512, 256, 256
NCORES = 8
BC = B // NCORES          # 8 sequences per core
G4 = 4 * H                # 1024 gate dims
STG = 32                  # recurrence steps per output staging block
BPB = 8                   # recurrence steps per PSUM bank (64 cols/step f32)

BF16 = mybir.dt.bfloat16
F32 = mybir.dt.float32
AF = mybir.ActivationFunctionType
ALU = mybir.AluOpType

# fp8e4 DoubleRow recurrence: Whh and h in e4m3, K=256 packed per matmul
# (8 burst MMs instead of 16; sigmoid(i,f,g) after 6). Weights/bias/Wih are
# prescaled x16 on host so e4m3 sees its normal range; the sigmoid's free
# input scale undoes it. h rides unscaled in e4m3.
FP8_DR = False   # DoubleRow fp8: LDWEIGHTS(256 strided rows) doesn't overlap
                 # the MM stream (123ns/MM vs bf16's 27) — net loss, keep off.
FP8 = mybir.dt.float8e4
FP8_NP = ml_dtypes.float8_e4m3fn
if FP8_DR:
    WHH_DT = FP8
    WHH_NP = FP8_NP
    PSC = 16.0
else:
    WHH_DT = mybir.dt.bfloat16
    WHH_NP = ml_dtypes.bfloat16
    PSC = 1.0
POOL_FC = False  # GPSIMD fc measured slower (q7 launch + sem latency puts
                 # cnew later than the DVE's own second slot).
# split sigmoid (f,g,i after 12 MMs + o after 16; act layout [f g i C o])
# vs one sigmoid over all 64 gate cols after 16 MMs (2 fewer ACT ops per
# period; act layout [f g i o C]).
SPLIT_SIGO = True
CPOS = 48 if SPLIT_SIGO else 64   # col of the C (cell-state) slot
OPOS = 64 if SPLIT_SIGO else 48   # col of the o-gate block

# gate chunk order (f, g, i, o): f,g,i are j=0..5 so sigmoid(f,g,i) can
# start after 12 of the 16 Whh matmuls; o last. The act tile layout is
# [f(0:16) g(16:32) i(32:48) C(48:64) o(64:80)] so the fused round-1 op
# reads in0=[f|g] and in1=[C|i] as simple 2-block strided views.
_PERM = np.concatenate([
    np.arange(H, 2 * H),        # f
    np.arange(2 * H, 3 * H),    # g
    np.arange(0, H),            # i
    np.arange(3 * H, 4 * H),    # o
])


def build_nc(t_steps=T):
    assert t_steps % STG == 0
    nb = t_steps // STG               # staging blocks
    npb = t_steps // BPB              # psum banks (per dir) over the run
    TB = t_steps * BC                 # (t,b) columns per k-half of xT

    nc = bacc.Bacc(None, target_bir_lowering=False)

    xt_d, wih_d, whh_d, bias_d = {}, {}, {}, {}
    for d in ("f", "b"):
        xt_d[d] = nc.dram_tensor(f"xt_{d}", [128, 2 * TB], BF16, kind="ExternalInput")
        wih_d[d] = nc.dram_tensor(f"wih_{d}", [128, 2048], BF16, kind="ExternalInput")
        whh_d[d] = nc.dram_tensor(f"whh_{d}", [128, 2048], WHH_DT, kind="ExternalInput")
        bias_d[d] = nc.dram_tensor(f"bias_{d}", [128, 1024], BF16, kind="ExternalInput")
    out_e = nc.dram_tensor("out", [128, t_steps * 4 * BC], BF16, kind="ExternalOutput")

    with tile.TileContext(nc) as tc:
        with (
            tc.tile_pool(name="big", bufs=1) as big,
            tc.tile_pool(name="work", bufs=6) as work,
            tc.tile_pool(name="stgp", bufs=3) as stgp,
            tc.tile_pool(name="bank_f", bufs=4, space=bass.MemorySpace.PSUM) as bkf,
            tc.tile_pool(name="bank_b", bufs=4, space=bass.MemorySpace.PSUM) as bkb,
        ):
            bkp = {"f": bkf, "b": bkb}
            xt, wih, whh, bias = {}, {}, {}, {}
            for d in ("f", "b"):
                xt[d] = big.tile([128, 2 * TB], BF16, tag=f"xt{d}", name=f"xt{d}")
                wih[d] = big.tile([128, 2048], BF16, tag=f"wih{d}", name=f"wih{d}")
                bias[d] = big.tile([128, 1024], BF16, tag=f"bias{d}", name=f"bias{d}")
                whh[d] = big.tile([128, 2048], WHH_DT, tag=f"whh{d}", name=f"whh{d}")
            # ones row-0 / zeros elsewhere: the bias matmul is a normal
            # K=128 matmul (bias lhsT has zero rows 1-127) so the PE never
            # reconfigures tile size mid-projection (a K=1 matmul between
            # K=128 ones breaks LDWEIGHTS/MM overlap, ~230ns each).
            ones = big.tile([128, BPB * BC], BF16, tag="ones", name="ones")
            nc.vector.memset(ones[:], 0.0)
            nc.vector.memset(ones[0:1, :], 1.0)
            zh = big.tile([128, 2 * BC], WHH_DT if FP8_DR else BF16, tag="zh", name="zh")
            nc.vector.memset(zh[:], 0.0)

            # PE pstate warmup: ~40 dep-free dummy matmuls on memset tiles
            # execute during the input-DMA window, ramping the PE out of its
            # 0.65GHz cold pstate before the first real projection.
            warm = bkf.tile([128, 512], F32, tag="bkf", name="warm")
            for _ in range(40):
                nc.tensor.matmul(
                    warm[0:64, :BC], ones[:, :64], ones[:, :BC],
                    start=True, stop=False, skip_group_check=True,
                )

            # xt arrives in per-64-step chunks, paced so descriptor
            # generation on the Sync queue stays off the prologue.
            XCH = 512 if TB >= 512 else TB
            nchunk = TB // XCH

            def emit_xt_chunk(c):
                for d in ("f", "b"):
                    for kk in (0, 1):
                        nc.sync.dma_start(
                            xt[d][:, kk * TB + c * XCH : kk * TB + (c + 1) * XCH],
                            xt_d[d][:, kk * TB + c * XCH : kk * TB + (c + 1) * XCH],
                        )

            # descriptor order = first-needed first (descriptor generation on
            # the Sync queue is ~650ns serial each): xt chunk0 + Wih feed the
            # first projection; whh feeds step 0's burst; bias before the
            # first bias-MM; chunk1 later.
            for kk in (0, 1):
                nc.sync.dma_start(
                    xt["f"][:, kk * TB : kk * TB + XCH],
                    xt_d["f"][:, kk * TB : kk * TB + XCH],
                )
            nc.sync.dma_start(wih["f"][:], wih_d["f"][:])
            nc.sync.dma_start(bias["f"][:], bias_d["f"][:])
            for kk in (0, 1):
                nc.sync.dma_start(
                    xt["b"][:, kk * TB : kk * TB + XCH],
                    xt_d["b"][:, kk * TB : kk * TB + XCH],
                )
            nc.sync.dma_start(wih["b"][:], wih_d["b"][:])
            nc.sync.dma_start(bias["b"][:], bias_d["b"][:])
            nc.sync.dma_start(whh["f"][:], whh_d["f"][:])
            nc.sync.dma_start(whh["b"][:], whh_d["b"][:])
            if nchunk > 1:
                emit_xt_chunk(1)

            # ---- projection: fill one PSUM bank (8 steps) of one dir ----
            # bank col layout: j*64 + toff*8 + b. Proj MM j writes [128,64]
            # contiguous; bias rides a K=1 matmul against a ones row.
            # Emission is paced ~one j-group (3 MMs) per dir per step so the
            # in-order PE never sees a 24-MM projection wall ahead of a
            # recurrence burst.
            banks = {"f": {}, "b": {}}

            # scheduler-pinning state: popped proj matmuls are sandwiched
            # between this step's recurrence matmuls and the next step's via
            # ordering-only deps, otherwise the Tile scheduler bunches a whole
            # bank's projection into one 1us PE wall in front of a burst.
            step_anchor = [None]   # last rec MM of current step
            pend_proj = []         # proj MMs that must precede next step's rec

            def emit_proj_j(d, blk, j):
                if j == 0:
                    banks[d][blk] = bkp[d].tile(
                        [128, 512], F32, tag=f"bk{d}", name=f"bk{d}"
                    )
                bk = banks[d][blk]
                t0 = blk * BPB
                mms = []
                for kk in (0, 1):
                    mms.append(nc.tensor.matmul(
                        bk[:, j * 64 : (j + 1) * 64],
                        wih[d][:, kk * 1024 + j * 128 : kk * 1024 + (j + 1) * 128],
                        xt[d][:, kk * TB + t0 * BC : kk * TB + (t0 + BPB) * BC],
                        start=(j == 0 and kk == 0),
                        stop=False,
                        skip_group_check=True,
                    ))
                mms.append(nc.tensor.matmul(
                    bk[:, j * 64 : (j + 1) * 64],
                    bias[d][:, j * 128 : (j + 1) * 128],
                    ones[:, :],
                    start=False,
                    stop=False,
                    skip_group_check=True,
                ))
                return mms

            proj_q = {"f": [], "b": []}

            def push_bank(d, blk):
                if blk < npb:
                    proj_q[d].extend((blk, j) for j in range(8))

            def pop_proj(d, n):
                for _ in range(n):
                    if not proj_q[d]:
                        return
                    blk, j = proj_q[d].pop(0)
                    mms = emit_proj_j(d, blk, j)
                    if step_anchor[0] is not None:
                        add_dep_helper(mms[0].ins, step_anchor[0].ins, sync=False,
                                       reason="pace proj after this step's burst")
                    pend_proj.extend(mms)

            # ---- recurrence ----
            stg_tiles = {}

            def stg_slot(u):
                return stg_tiles[u // STG], (u % STG) * 4 * BC

            hqst = {}

            def _pin(mm, first):
                if first and pend_proj:
                    for p in pend_proj:
                        add_dep_helper(mm.ins, p.ins, sync=False,
                                       reason="pace proj before next burst")
                    pend_proj.clear()
                step_anchor[0] = mm

            def emit_whh(d, doff, t):
                bk = banks[d][t // BPB]
                toff = t % BPB
                if FP8_DR:
                    # DoubleRow: K=256 packed, one matmul per gate chunk j;
                    # sigmoid(i,f,g) can start after 6 of the 8.
                    prevq = zh[:] if t == 0 else hqst[d][:]
                    pv = prevq.rearrange("p (kk b) -> p kk b", kk=2)
                    wv = whh[d][:].rearrange("p (kk g) -> p kk g", kk=2)
                    for n, j in enumerate((0, 1, 2, 3, 4, 5, 6, 7)):
                        mm = nc.tensor.matmul(
                            bk[:, j * 64 + toff * 8 : j * 64 + toff * 8 + 8],
                            wv[:, :, j * 128 : (j + 1) * 128],
                            pv,
                            start=False,
                            stop=True,
                            perf_mode=mybir.MatmulPerfMode.DoubleRow,
                            skip_group_check=True,
                        )
                        _pin(mm, d == "f" and n == 0)
                    return
                if t == 0:
                    prev = zh[:]
                else:
                    st, off = stg_slot(t - 1)
                    prev = st[:, off + doff : off + doff + 2 * BC]
                # [kk0 j0-5][kk1 j0-5][kk0 j6-7][kk1 j6-7]; stop on each
                # region's last (kk1) matmul.
                first = True
                for kk, js in ((0, range(6)), (1, range(6)), (0, (6, 7)), (1, (6, 7))):
                    rhs = prev[:, kk * BC : (kk + 1) * BC]
                    for j in js:
                        mm = nc.tensor.matmul(
                            bk[:, j * 64 + toff * 8 : j * 64 + toff * 8 + 8],
                            whh[d][:, kk * 1024 + j * 128 : kk * 1024 + (j + 1) * 128],
                            rhs,
                            start=False,
                            stop=(kk == 1),
                            skip_group_check=True,
                        )
                        _pin(mm, d == "f" and first)
                        first = False

            EWT = BF16
            last_hm0 = {}
            # per-dir double-buffered act tiles [f g i C o]; the C slot
            # (cols 48:64) carries the half-cell state C = c/2 across steps.
            actb = {}
            for d in ("f", "b"):
                actb[d] = [
                    big.tile([128, 80], EWT, tag=f"act{d}{k}", name=f"act{d}{k}")
                    for k in (0, 1)
                ]
                for k in (0, 1):
                    nc.vector.memset(actb[d][k][:, CPOS : CPOS + 16], 0.0)

            def emit_ew(d, doff, t):
                bk = banks[d][t // BPB]
                toff = t % BPB
                st, off = stg_slot(t)
                v = bk[:].rearrange("p (j tb) -> p j tb", j=8)
                A = actb[d][t % 2]
                B = actb[d][(t + 1) % 2]
                if SPLIT_SIGO:
                    nc.scalar.activation(
                        A[:, 0:48], v[:, 0:6, toff * 8 : (toff + 1) * 8],
                        AF.Sigmoid, scale=1.0 / PSC,
                    )
                    nc.scalar.activation(
                        A[:, OPOS : OPOS + 16], v[:, 6:8, toff * 8 : (toff + 1) * 8],
                        AF.Sigmoid, scale=1.0 / PSC,
                    )
                else:
                    nc.scalar.activation(
                        A[:, 0:64], v[:, 0:8, toff * 8 : (toff + 1) * 8],
                        AF.Sigmoid, scale=1.0 / PSC,
                    )
                # fused round-1: out = [sig(f)*C | (sig(2g)-0.5)*sig(i)]
                fu = work.tile([128, 4 * BC], EWT, tag=f"fu{d}", name=f"fu{d}")
                in0 = A[:, 0:32].rearrange("p (s n) -> p s n", s=2)
                if SPLIT_SIGO:
                    in1 = A[:].rearrange("p (s n) -> p s n", n=16)[:, 3:1:-1, :]
                else:
                    in1 = A[:].rearrange("p (s n) -> p s n", n=16)[:, 4:0:-2, :]
                nc.vector._custom_dve(
                    FG_PROD, out=fu[:].rearrange("p (s n) -> p s n", s=2),
                    in0=in0, in1=in1, s0=0.5, s1=float(2 * BC),
                )
                # C(t) = fc + tmp', written into the other buffer's C slot
                add_ins = nc.vector.tensor_add(
                    B[:, CPOS : CPOS + 16], fu[:, : 2 * BC], fu[:, 2 * BC :]
                )
                other = "b" if d == "f" else "f"
                if last_hm0.get(other) is not None:
                    add_dep_helper(add_ins.ins, last_hm0[other].ins, sync=False,
                                   reason="keep h-mul ahead of other chain's c-add")
                th = work.tile([128, 2 * BC], EWT, tag=f"th{d}", name=f"th{d}")
                nc.scalar.activation(th[:], B[:, CPOS : CPOS + 16], AF.Tanh, scale=2.0)
                if FP8_DR:
                    # on-path: fp8 h for the next burst; off-path: bf16 copy
                    # into the output staging tile.
                    hq = work.tile([128, 2 * BC], FP8, tag=f"hq{d}", name=f"hq{d}",
                                   bufs=3)
                    hmq = nc.vector.tensor_mul(hq[:], A[:, OPOS : OPOS + 16], th[:])
                    hqst[d] = hq
                    nc.vector.tensor_mul(
                        st[:, off + doff : off + doff + 2 * BC],
                        A[:, OPOS : OPOS + 16], th[:],
                    )
                    last_hm0[d] = hmq
                    return
                hm0 = nc.vector.tensor_mul(
                    st[:, off + doff : off + doff + BC],
                    A[:, OPOS : OPOS + BC],
                    th[:, :BC],
                )
                nc.vector.tensor_mul(
                    st[:, off + doff + BC : off + doff + 2 * BC],
                    A[:, OPOS + BC : OPOS + 2 * BC],
                    th[:, BC:],
                )
                last_hm0[d] = hm0

            # upfront: bank 0 per dir fully; banks 1-2 drain through the
            # paced queue during the first steps (deadline: bank k's first
            # recurrence matmul is at t=8k, queue drains ≥1 j-group/step).
            for d in ("f", "b"):
                for j in range(8):
                    emit_proj_j(d, 0, j)
            for d in ("f", "b"):
                push_bank(d, 1)
                push_bank(d, 2)

            stg_tiles[0] = stgp.tile([128, STG * 4 * BC], BF16, tag="stg", name="stg")
            for t in range(t_steps):
                if t % BPB == 2:
                    push_bank("f", t // BPB + 3)
                if t % BPB == 4:
                    push_bank("b", t // BPB + 3)
                if t % 64 == 24 and 2 <= t // 64 + 1 < nchunk:
                    emit_xt_chunk(t // 64 + 1)
                if t % STG == 0 and t > 0:
                    stg_tiles[t // STG] = stgp.tile(
                        [128, STG * 4 * BC], BF16, tag="stg", name="stg"
                    )
                emit_whh("f", 0, t)
                if t >= 1:
                    emit_ew("b", 2 * BC, t - 1)
                    if t % STG == 0:
                        blk = t // STG - 1
                        nc.sync.dma_start(
                            out_e[:, blk * STG * 4 * BC : (blk + 1) * STG * 4 * BC],
                            stg_tiles[blk][:],
                        )
                emit_whh("b", 2 * BC, t)
                emit_ew("f", 0, t)
                catchup = 2 if (len(proj_q["f"]) + len(proj_q["b"])) > 16 else 1
                pop_proj("f", catchup)
                pop_proj("b", catchup)
            emit_ew("b", 2 * BC, t_steps - 1)
            blk = nb - 1
            nc.sync.dma_start(
                out_e[:, blk * STG * 4 * BC : (blk + 1) * STG * 4 * BC],
                stg_tiles[blk][:],
            )

    nc.compile()
    return nc


def _prep_core(xs, Wih, Whh, bih, bhh, t_steps):
    """Host-side layout prep for one core, one direction.

    xs: [BC, t, D] f32 (already reversed for the backward direction).
    Returns dict of device arrays.
    """
    TB = t_steps * BC
    Wp = Wih[_PERM].astype(np.float32).copy()   # [1024, 256]
    Wh = Whh[_PERM].astype(np.float32).copy()
    bsum = (bih + bhh)[_PERM].astype(np.float32).copy()
    # tanh(g) rides the wide sigmoid: pre-scale g rows (perm positions
    # H:2H under the (f,g,i,o) chunk order) x2
    Wp[H : 2 * H] *= 2.0
    Wh[H : 2 * H] *= 2.0
    bsum[H : 2 * H] *= 2.0
    # fp8 prescale (undone by the sigmoid input scale)
    Wp *= PSC
    Wh *= PSC
    bsum *= PSC

    def wt_layout(W, dtype=ml_dtypes.bfloat16):  # [4H, 256] -> [128, 2048] lhsT
        WT = W.T.reshape(2, 128, G4).transpose(1, 0, 2).reshape(128, 2 * G4)
        return np.ascontiguousarray(WT).astype(dtype)

    xT = (
        xs.transpose(2, 1, 0)                   # [256, t, BC]
        .reshape(2, 128, TB)
        .transpose(1, 0, 2)
        .reshape(128, 2 * TB)
    )
    bz = np.zeros((128, 1024), np.float32)
    bz[0, :] = bsum
    return {
        "xt": np.ascontiguousarray(xT).astype(ml_dtypes.bfloat16),
        "wih": wt_layout(Wp),
        "whh": wt_layout(Wh, WHH_NP),
        "bias": bz.astype(ml_dtypes.bfloat16),
    }


_NC_CACHE = {}


def _get_nc(t_steps):
    if t_steps not in _NC_CACHE:
        _NC_CACHE[t_steps] = build_nc(t_steps)
    return _NC_CACHE[t_steps]


def kernel(x, input_length, Wih_f, Whh_f, bih_f, bhh_f, Wih_b, Whh_b, bih_b, bhh_b,
           t_steps=T, _want_trace=False):
    x = np.asarray(x, np.float32)
    lens = np.asarray(input_length).astype(np.int64)
    L = t_steps
    tt = np.arange(L)

    nc = _get_nc(t_steps)

    in_maps = []
    for c in range(NCORES):
        bs = slice(c * BC, (c + 1) * BC)
        xs = x[bs, :L]
        ls = lens[bs]
        inv_idx = L - 1 - ((L - ls[:, None] + tt[None, :]) % L)       # [BC, L]
        xn = np.take_along_axis(xs, inv_idx[:, :, None], axis=1)
        pf = _prep_core(xs, Wih_f, Whh_f, bih_f, bhh_f, L)
        pb = _prep_core(xn, Wih_b, Whh_b, bih_b, bhh_b, L)
        in_maps.append(
            {
                "xt_f": pf["xt"], "wih_f": pf["wih"], "whh_f": pf["whh"], "bias_f": pf["bias"],
                "xt_b": pb["xt"], "wih_b": pb["wih"], "whh_b": pb["whh"], "bias_b": pb["bias"],
            }
        )

    kw = {}
    if _want_trace:
        kw = dict(trace=True)
    res = run_bass_kernel_spmd(nc, in_maps, core_ids=list(range(NCORES)), **kw)

    outs = []
    for c in range(NCORES):
        bs = slice(c * BC, (c + 1) * BC)
        ls = lens[bs]
        arr = np.asarray(res.results[c]["out"]).astype(np.float32)
        arr = arr.reshape(128, L, 4, BC)
        fwd = arr[:, :, 0:2, :].transpose(3, 1, 2, 0).reshape(BC, L, 2 * 128)
        bwd = arr[:, :, 2:4, :].transpose(3, 1, 2, 0).reshape(BC, L, 2 * 128)
        bwd_idx = np.clip(ls[:, None] - 1 - tt[None, :], 0, L - 1)
        bwd_g = np.take_along_axis(bwd, bwd_idx[:, :, None], axis=1)
        o = np.concatenate([fwd, bwd_g], axis=-1)
        mask = (tt[None, :] < ls[:, None])[:, :, None]
        outs.append(np.where(mask, o, 0.0).astype(np.float32))
    full = np.concatenate(outs, axis=0)
    if _want_trace:
        return full, res
    return full
